# revision 1
# baseline (speedup 1.0000x reference)
"""CrossLinearAttention Trainium2 kernel (8 NeuronCores, SPMD).

Problem: b=4, n1=n2=8192, dim=256, 8 heads x 64 dim_head.
  q = x @ Wq.T                     (rotary 2D on q)
  k, v = split(z @ Wkv.T)          (LayerNorm per head-dim on k, v; rotary on k)
  dots = k^T v / n2 per (b, h);  out = (q @ dots) @ Wo.T + bo

Sharding: flatten (b, n) -> 16384 rows; core c owns rows [2048c, 2048(c+1)).
Each core's shard lies inside one batch element (b = c // 2); the z-side
partial k^T v is summed with a tiny AllReduce between core pairs
{0,1},{2,3},{4,5},{6,7}.

Device math (per core, fp32r matmuls at full PE rate):
  - Host pre-transposes weights; Wkv.T is mean-centered per 64-col head
    block so the projection emits k - mean(k) directly; 1/n2 is folded
    into Wq.T (exact, 2^-13).
  - LayerNorm scale rstd is folded into the v operand of dots
    (v'' = vsub * rstd_k * rstd_v), so k only needs the rotary.
  - Rotary sin/cos tables are host-precomputed from x_pos/z_pos
    ([rows, 32]: 16 x-freqs + 16 y-freqs), ACT Sin on device is only
    accurate near [-pi, pi] while phases reach 64 rad.
  - Activations are transposed on the PE (fp32r transpose mode); q is
    rotated row-major then transposed for the q@dots and out-proj
    contractions.
"""

import sys

sys.path.insert(0, "/opt/trn_rl_repo")

from contextlib import ExitStack

import numpy as np

import concourse.bass as bass
import concourse.tile as tile
from concourse import bacc, mybir
from concourse.bass import ts
from concourse.bass_utils import run_bass_kernel_spmd
from concourse.masks import make_identity

F32 = mybir.dt.float32
F32R = mybir.dt.float32r
ALU = mybir.AluOpType
AX = mybir.AxisListType
AF = mybir.ActivationFunctionType

B, N1, DIM = 4, 8192, 256
H, DH = 8, 64
INNER = H * DH  # 512
NCORES = 8
ROWS = (B * N1) // NCORES  # 2048 rows per core
NT = ROWS // 128  # 16 tiles of 128 rows
EPS = 1e-5

_nc_cache = {}
DEBUG = False


def _view(ap, offset, dims):
    """AP view with explicit free dims; keeps the partition dim."""
    return bass.AP(
        tensor=ap.tensor,
        offset=ap.offset + offset,
        ap=[list(ap.ap[0])] + [list(d) for d in dims],
    )


def _rotary(nc, out_f32r, src_ps, cos_t, sin_t, kcs, krs):
    """out = 2D-rotary(src) row-major: src/out [128, 512] = 8 heads x
    (4 quads x 16); cos/sin [128, 32] = (x|y) x 16 freqs.
    kcs/krs are [128, 512] F32 scratch. src_ps may be PSUM (DVE reads it).
    """
    for a in (0, 1):  # 0: x-axis (head cols 0:32), 1: y-axis (32:64)
        nc.vector.tensor_tensor(
            _view(kcs, a * 32, [[64, 8], [16, 2], [1, 16]]),
            _view(src_ps, a * 32, [[64, 8], [16, 2], [1, 16]]),
            _view(cos_t, a * 16, [[0, 8], [0, 2], [1, 16]]),
            op=ALU.mult,
        )
        nc.vector.tensor_tensor(
            _view(krs, a * 32, [[64, 8], [16, 2], [1, 16]]),
            _view(src_ps, a * 32 + 16, [[64, 8], [-16, 2], [1, 16]]),
            _view(sin_t, a * 16, [[0, 8], [0, 2], [1, 16]]),
            op=ALU.mult,
        )
    # even quads: cos-part - sin-part; odd: +
    nc.gpsimd.tensor_tensor(
        _view(out_f32r, 0, [[64, 8], [32, 2], [1, 16]]),
        _view(kcs, 0, [[64, 8], [32, 2], [1, 16]]),
        _view(krs, 0, [[64, 8], [32, 2], [1, 16]]),
        op=ALU.subtract,
    )
    nc.gpsimd.tensor_tensor(
        _view(out_f32r, 16, [[64, 8], [32, 2], [1, 16]]),
        _view(kcs, 16, [[64, 8], [32, 2], [1, 16]]),
        _view(krs, 16, [[64, 8], [32, 2], [1, 16]]),
        op=ALU.add,
    )


def build_nc(triv_gamma: bool):
    if triv_gamma in _nc_cache:
        return _nc_cache[triv_gamma]
    nc = bacc.Bacc(trn_type="TRN2", num_devices=NCORES, debug=False)

    x_d = nc.dram_tensor("x", [ROWS, DIM], F32R, kind="ExternalInput").ap()
    z_d = nc.dram_tensor("z", [ROWS, DIM], F32R, kind="ExternalInput").ap()
    wq_d = nc.dram_tensor("wq", [DIM, INNER], F32R, kind="ExternalInput").ap()
    wkv_d = nc.dram_tensor("wkv", [DIM, 2 * INNER], F32R, kind="ExternalInput").ap()
    wo_d = nc.dram_tensor("wo", [INNER, DIM], F32R, kind="ExternalInput").ap()
    bo_d = nc.dram_tensor("bo", [1, DIM], F32R, kind="ExternalInput").ap()
    cosq_d = nc.dram_tensor("cosq", [ROWS, 32], F32, kind="ExternalInput").ap()
    sinq_d = nc.dram_tensor("sinq", [ROWS, 32], F32, kind="ExternalInput").ap()
    cosk_d = nc.dram_tensor("cosk", [ROWS, 32], F32, kind="ExternalInput").ap()
    sink_d = nc.dram_tensor("sink", [ROWS, 32], F32, kind="ExternalInput").ap()
    if not triv_gamma:
        gk_d = nc.dram_tensor("gk", [1, INNER], F32, kind="ExternalInput").ap()
        bk_d = nc.dram_tensor("bk", [1, INNER], F32, kind="ExternalInput").ap()
        gv_d = nc.dram_tensor("gv", [1, INNER], F32, kind="ExternalInput").ap()
        bv_d = nc.dram_tensor("bv", [1, INNER], F32, kind="ExternalInput").ap()
    y_d = nc.dram_tensor("y", [ROWS, DIM], F32, kind="ExternalOutput").ap()
    cc_in = nc.dram_tensor("cc_in", [128, 512], F32)
    cc_out = nc.dram_tensor("cc_out", [128, 512], F32)
    if DEBUG:
        dbg_krot = nc.dram_tensor("dbg_krot", [ROWS, INNER], F32, kind="ExternalOutput").ap()
        dbg_v2 = nc.dram_tensor("dbg_v2", [ROWS, INNER], F32, kind="ExternalOutput").ap()
        dbg_dots = nc.dram_tensor("dbg_dots", [128, 512], F32, kind="ExternalOutput").ap()
        dbg_blk = nc.dram_tensor("dbg_blk", [128, 512], F32, kind="ExternalOutput").ap()
        dbg_qrot = nc.dram_tensor("dbg_qrot", [ROWS, INNER], F32, kind="ExternalOutput").ap()

    with tile.TileContext(nc) as tc, ExitStack() as ctx:
        consts = ctx.enter_context(tc.tile_pool(name="consts", bufs=1))

        ident_f = consts.tile([128, 128], F32)
        make_identity(nc, ident_f)
        ident = consts.tile([128, 128], F32R)
        nc.vector.tensor_copy(ident, ident_f)

        eps_t = consts.tile([128, 1], F32)
        nc.vector.memset(eps_t, EPS)
        ones_f = consts.tile([1, 128], F32)
        nc.vector.memset(ones_f, 1.0)
        ones_r = consts.tile([1, 128], F32R)
        nc.vector.tensor_copy(ones_r, ones_f)
        zeros_t = consts.tile([128, 512], F32)
        nc.vector.memset(zeros_t, 0.0)

        wq_t = []
        for kc in range(2):
            t = consts.tile([128, INNER], F32R, tag=f"wq{kc}")
            nc.sync.dma_start(t, wq_d[ts(kc, 128), :])
            wq_t.append(t)
        wkv_t = []
        for kc in range(2):
            t = consts.tile([128, 2 * INNER], F32R, tag=f"wkv{kc}")
            nc.sync.dma_start(t, wkv_d[ts(kc, 128), :])
            wkv_t.append(t)
        wo_t = []
        for c in range(4):
            t = consts.tile([128, DIM], F32R, tag=f"wo{c}")
            nc.sync.dma_start(t, wo_d[ts(c, 128), :])
            wo_t.append(t)
        bo_t = consts.tile([1, DIM], F32R)
        nc.sync.dma_start(bo_t, bo_d)
        if not triv_gamma:
            gb_t = {}
            for name, d in (("gk", gk_d), ("bk", bk_d), ("gv", gv_d), ("bv", bv_d)):
                t = consts.tile([128, INNER], F32, tag=name)
                nc.sync.dma_start(t, _view(d, 0, [[1, INNER]]).partition_broadcast(128))
                gb_t[name] = t

        # ---------------- phase 1: z side -> dots ----------------
        with ExitStack() as p1:
            zin = p1.enter_context(tc.tile_pool(name="zin", bufs=3))
            zt_pp = p1.enter_context(tc.tile_pool(name="zt_pp", bufs=2, space="PSUM"))
            zt_sp = p1.enter_context(tc.tile_pool(name="zt_sp", bufs=2))
            kv_pp = p1.enter_context(tc.tile_pool(name="kv_pp", bufs=1, space="PSUM"))
            dots_pp = p1.enter_context(
                tc.tile_pool(name="dots_pp", bufs=1, space="PSUM")
            )
            elem = p1.enter_context(tc.tile_pool(name="elem", bufs=2))
            tabs = p1.enter_context(tc.tile_pool(name="tabs", bufs=3))
            stats = p1.enter_context(tc.tile_pool(name="stats", bufs=3))

            # one PSUM tile (= one bank) per head pair: two accumulation
            # groups must NOT share a bank (corrupts accumulation)
            dots_tiles = []
            for p in range(4):
                d_t = dots_pp.tile([128, 256], F32, tag=f"dots{p}", name=f"dots{p}")
                dots_tiles.append(d_t)

            for t in range(NT):
                z_t = zin.tile([128, DIM], F32R, tag="z")
                nc.sync.dma_start(z_t, z_d[ts(t, 128), :])
                cosk_t = tabs.tile([128, 32], F32, tag="cosk")
                nc.sync.dma_start(cosk_t, cosk_d[ts(t, 128), :])
                sink_t = tabs.tile([128, 32], F32, tag="sink")
                nc.sync.dma_start(sink_t, sink_d[ts(t, 128), :])

                zT_ps = zt_pp.tile([128, DIM], F32R, tag="zT_ps")
                for kc in range(2):
                    nc.tensor.transpose(
                        zT_ps[:, ts(kc, 128)], z_t[:, ts(kc, 128)], ident
                    )
                zT = zt_sp.tile([128, DIM], F32R, tag="zT")
                nc.scalar.copy(zT, zT_ps)

                kvk_ps = kv_pp.tile([128, INNER], F32, tag="kvk")
                kvv_ps = kv_pp.tile([128, INNER], F32, tag="kvv")
                for kc in range(2):
                    nc.tensor.matmul(
                        kvk_ps,
                        zT[:, ts(kc, 128)],
                        wkv_t[kc][:, 0:INNER],
                        start=(kc == 0),
                        stop=(kc == 1),
                    )
                for kc in range(2):
                    nc.tensor.matmul(
                        kvv_ps,
                        zT[:, ts(kc, 128)],
                        wkv_t[kc][:, INNER : 2 * INNER],
                        start=(kc == 0),
                        stop=(kc == 1),
                    )

                sq = elem.tile([128, 2 * INNER], F32, tag="sq")
                nc.scalar.activation(sq[:, 0:INNER], kvk_ps, AF.Square)
                nc.scalar.activation(sq[:, INNER : 2 * INNER], kvv_ps, AF.Square)
                sums = stats.tile([128, 16], F32, tag="sums")
                nc.vector.reduce_sum(
                    sums, _view(sq, 0, [[64, 16], [1, 64]]), axis=AX.X
                )
                std = stats.tile([128, 16], F32, tag="std")
                nc.scalar.activation(
                    std, sums, AF.Sqrt, scale=1.0 / DH, bias=eps_t[:, 0:1]
                )
                rstd = stats.tile([128, 16], F32, tag="rstd")
                nc.vector.reciprocal(rstd, std)

                krot = elem.tile([128, INNER], F32R, tag="krot")
                v2 = elem.tile([128, INNER], F32R, tag="v2")
                kcs = elem.tile([128, INNER], F32, tag="kcs")
                krs = elem.tile([128, INNER], F32, tag="krs")
                if triv_gamma:
                    rkv = stats.tile([128, 8], F32, tag="rkv")
                    nc.vector.tensor_tensor(
                        rkv, rstd[:, 0:8], rstd[:, 8:16], op=ALU.mult
                    )
                    nc.vector.tensor_tensor(
                        _view(v2, 0, [[64, 8], [1, 64]]),
                        _view(kvv_ps, 0, [[64, 8], [1, 64]]),
                        _view(rkv, 0, [[1, 8], [0, 64]]),
                        op=ALU.mult,
                    )
                    _rotary(nc, krot, kvk_ps, cosk_t, sink_t, kcs, krs)
                else:
                    kg = elem.tile([128, INNER], F32, tag="kg")
                    nc.vector.tensor_tensor(
                        _view(kg, 0, [[64, 8], [1, 64]]),
                        _view(kvk_ps, 0, [[64, 8], [1, 64]]),
                        _view(rstd, 0, [[1, 8], [0, 64]]),
                        op=ALU.mult,
                    )
                    nc.vector.tensor_tensor(kg, kg, gb_t["gk"], op=ALU.mult)
                    nc.gpsimd.tensor_tensor(kg, kg, gb_t["bk"], op=ALU.add)
                    _rotary(nc, krot, kg, cosk_t, sink_t, kcs, krs)
                    vg = elem.tile([128, INNER], F32, tag="vg")
                    nc.vector.tensor_tensor(
                        _view(vg, 0, [[64, 8], [1, 64]]),
                        _view(kvv_ps, 0, [[64, 8], [1, 64]]),
                        _view(rstd, 8, [[1, 8], [0, 64]]),
                        op=ALU.mult,
                    )
                    nc.vector.tensor_tensor(vg, vg, gb_t["gv"], op=ALU.mult)
                    nc.vector.tensor_tensor(v2, vg, gb_t["bv"], op=ALU.add)

                for p in range(4):
                    nc.tensor.matmul(
                        dots_tiles[p],
                        krot[:, ts(p, 128)],
                        v2[:, (p // 2) * 256 : (p // 2) * 256 + 256],
                        start=(t == 0),
                        stop=(t == NT - 1),
                    )
                if DEBUG:
                    nc.sync.dma_start(dbg_krot[ts(t, 128), :], krot.bitcast(F32))
                    nc.sync.dma_start(dbg_v2[ts(t, 128), :], v2.bitcast(F32))

            # useful block of pair p sits at cols (p%2)*128 of its tile
            dots_sb = consts.tile([128, 512], F32)
            for p in range(4):
                off = (p % 2) * 128
                nc.vector.tensor_copy(
                    dots_sb[:, ts(p, 128)], dots_tiles[p][:, off : off + 128]
                )

        nc.sync.dma_start(cc_in.ap(), dots_sb)
        nc.gpsimd.collective_compute(
            "AllReduce",
            ALU.add,
            replica_groups=[[0, 1], [2, 3], [4, 5], [6, 7]],
            ins=[cc_in.ap()],
            outs=[cc_out.ap()],
        )
        dots_rd = consts.tile([128, 512], F32)
        nc.sync.dma_start(dots_rd, cc_out.ap())

        # block-diagonal per-pair lhsT for attn: zero cross-head quadrants
        blk = consts.tile([128, 512], F32R)
        nc.vector.tensor_copy(blk, zeros_t)
        for p in range(4):
            nc.vector.tensor_copy(
                blk[0:64, p * 128 : p * 128 + 64],
                dots_rd[0:64, p * 128 : p * 128 + 64],
            )
            nc.vector.tensor_copy(
                blk[64:128, p * 128 + 64 : p * 128 + 128],
                dots_rd[64:128, p * 128 + 64 : p * 128 + 128],
            )
        if DEBUG:
            nc.sync.dma_start(dbg_dots, dots_sb)
            nc.sync.dma_start(dbg_blk, blk.bitcast(F32))

        # ---------------- phase 2: x side -> y ----------------
        with ExitStack() as p2:
            xin = p2.enter_context(tc.tile_pool(name="xin", bufs=3))
            xt_pp = p2.enter_context(tc.tile_pool(name="xt_pp", bufs=1, space="PSUM"))
            xt_sp = p2.enter_context(tc.tile_pool(name="xt_sp", bufs=2))
            q_pp = p2.enter_context(tc.tile_pool(name="q_pp", bufs=2, space="PSUM"))
            qt_pp = p2.enter_context(tc.tile_pool(name="qt_pp", bufs=1, space="PSUM"))
            at_pp = p2.enter_context(tc.tile_pool(name="at_pp", bufs=1, space="PSUM"))
            y_pp = p2.enter_context(tc.tile_pool(name="y_pp", bufs=1, space="PSUM"))
            elem2 = p2.enter_context(tc.tile_pool(name="elem2", bufs=2))
            tabs2 = p2.enter_context(tc.tile_pool(name="tabs2", bufs=3))
            outp = p2.enter_context(tc.tile_pool(name="outp", bufs=3))

            for drt in range(NT // 2):
                qT_ps = qt_pp.tile([128, 1024], F32R, tag="qT_ps")
                for j in range(2):
                    rt = 2 * drt + j
                    x_t = xin.tile([128, DIM], F32R, tag="x")
                    nc.sync.dma_start(x_t, x_d[ts(rt, 128), :])
                    cosq_t = tabs2.tile([128, 32], F32, tag="cosq")
                    nc.sync.dma_start(cosq_t, cosq_d[ts(rt, 128), :])
                    sinq_t = tabs2.tile([128, 32], F32, tag="sinq")
                    nc.sync.dma_start(sinq_t, sinq_d[ts(rt, 128), :])

                    xT_ps = xt_pp.tile([128, DIM], F32R, tag="xT_ps")
                    for kc in range(2):
                        nc.tensor.transpose(
                            xT_ps[:, ts(kc, 128)], x_t[:, ts(kc, 128)], ident
                        )
                    xT = xt_sp.tile([128, DIM], F32R, tag="xT")
                    nc.scalar.copy(xT, xT_ps)

                    q_ps = q_pp.tile([128, INNER], F32, tag="q_ps")
                    for kc in range(2):
                        nc.tensor.matmul(
                            q_ps,
                            xT[:, ts(kc, 128)],
                            wq_t[kc],
                            start=(kc == 0),
                            stop=(kc == 1),
                        )

                    qrot = elem2.tile([128, INNER], F32R, tag="qrot")
                    qcs = elem2.tile([128, INNER], F32, tag="qcs")
                    qrs = elem2.tile([128, INNER], F32, tag="qrs")
                    _rotary(nc, qrot, q_ps, cosq_t, sinq_t, qcs, qrs)
                    if DEBUG:
                        nc.sync.dma_start(dbg_qrot[ts(rt, 128), :], qrot.bitcast(F32))

                    for c in range(4):
                        nc.tensor.transpose(
                            qT_ps[:, c * 256 + j * 128 : c * 256 + (j + 1) * 128],
                            qrot[:, ts(c, 128)],
                            ident,
                        )

                qT_sb = elem2.tile([128, 1024], F32R, tag="qT_sb")
                nc.scalar.copy(qT_sb, qT_ps)

                attn_ps = at_pp.tile([128, 1024], F32, tag="attn_ps")
                for p in range(4):
                    nc.tensor.matmul(
                        attn_ps[:, ts(p, 256)],
                        blk[:, ts(p, 128)],
                        qT_sb[:, ts(p, 256)],
                        start=True,
                        stop=True,
                    )
                attnT = elem2.tile([128, 1024], F32R, tag="attnT")
                nc.scalar.copy(attnT, attn_ps)

                for j in range(2):
                    rt = 2 * drt + j
                    y_ps = y_pp.tile([128, DIM], F32, tag="y_ps")
                    for c in range(4):
                        nc.tensor.matmul(
                            y_ps,
                            attnT[:, c * 256 + j * 128 : c * 256 + (j + 1) * 128],
                            wo_t[c],
                            start=(c == 0),
                            stop=False,
                        )
                    nc.tensor.matmul(y_ps, ones_r, bo_t, start=False, stop=True)
                    y_sb = outp.tile([128, DIM], F32, tag="y_sb")
                    nc.vector.tensor_copy(y_sb, y_ps)
                    nc.sync.dma_start(y_d[ts(rt, 128), :], y_sb)

    nc.compile()
    _nc_cache[triv_gamma] = nc
    return nc


def _tables(pos):
    """pos [rows, 2] -> cos/sin [rows, 32] (16 x-freqs | 16 y-freqs), f32."""
    rdim = DH // 2  # 32
    inv_freq = (
        1.0 / (10000.0 ** (np.arange(0, rdim, 2, dtype=np.float32) / rdim))
    ).astype(np.float32)  # [16]
    t = pos.astype(np.float32) * np.float32(64.0)  # SCALE / MIN_FREQ
    f = np.concatenate(
        [t[:, 0:1] * inv_freq, t[:, 1:2] * inv_freq], axis=1
    )  # [rows, 32]
    return np.cos(f).astype(np.float32), np.sin(f).astype(np.float32)


def _prepare(
    x, z, x_pos, z_pos, Wq, Wkv, k_gamma, k_beta, v_gamma, v_beta, Wo, bo
):
    """Host prep: returns (nc, in_maps) ready for run_bass_kernel_spmd."""
    x = np.asarray(x)
    z = np.asarray(z)
    xf = np.ascontiguousarray(x.reshape(B * N1, DIM), dtype=np.float32)
    zf = np.ascontiguousarray(z.reshape(B * N1, DIM), dtype=np.float32)
    xpf = np.asarray(x_pos).reshape(B * N1, 2)
    zpf = np.asarray(z_pos).reshape(B * N1, 2)

    wqT = np.ascontiguousarray(np.asarray(Wq).T / np.float32(N1)).astype(np.float32)
    wkvT = np.asarray(Wkv).T.astype(np.float32)  # [256, 1024]
    wkv_c = wkvT.reshape(DIM, 16, DH)
    wkvT = np.ascontiguousarray(
        (wkv_c - wkv_c.mean(axis=2, keepdims=True)).reshape(DIM, 2 * INNER)
    ).astype(np.float32)
    woT = np.ascontiguousarray(np.asarray(Wo).T).astype(np.float32)  # [512, 256]
    bo_r = np.ascontiguousarray(np.asarray(bo).reshape(1, DIM)).astype(np.float32)

    cq, sq_ = _tables(xpf)
    ck, sk_ = _tables(zpf)

    triv = (
        np.all(np.asarray(k_gamma) == 1.0)
        and np.all(np.asarray(k_beta) == 0.0)
        and np.all(np.asarray(v_gamma) == 1.0)
        and np.all(np.asarray(v_beta) == 0.0)
    )
    nc = build_nc(bool(triv))

    in_maps = []
    for c in range(NCORES):
        lo, hi = c * ROWS, (c + 1) * ROWS
        m = {
            "x": xf[lo:hi],
            "z": zf[lo:hi],
            "wq": wqT,
            "wkv": wkvT,
            "wo": woT,
            "bo": bo_r,
            "cosq": np.ascontiguousarray(cq[lo:hi]),
            "sinq": np.ascontiguousarray(sq_[lo:hi]),
            "cosk": np.ascontiguousarray(ck[lo:hi]),
            "sink": np.ascontiguousarray(sk_[lo:hi]),
        }
        if not triv:
            m["gk"] = np.ascontiguousarray(np.tile(np.asarray(k_gamma), H)[None, :]).astype(np.float32)
            m["bk"] = np.ascontiguousarray(np.tile(np.asarray(k_beta), H)[None, :]).astype(np.float32)
            m["gv"] = np.ascontiguousarray(np.tile(np.asarray(v_gamma), H)[None, :]).astype(np.float32)
            m["bv"] = np.ascontiguousarray(np.tile(np.asarray(v_beta), H)[None, :]).astype(np.float32)
        in_maps.append(m)
    return nc, in_maps


def kernel(**inputs):
    nc, in_maps = _prepare(**inputs)
    res = run_bass_kernel_spmd(nc, in_maps, list(range(NCORES)))
    y = np.concatenate([res.results[c]["y"] for c in range(NCORES)], axis=0)
    return y.reshape(B, N1, DIM).astype(np.float32)



# revision 6
# speedup vs baseline: 1.5131x; 1.5131x over previous
"""CrossLinearAttention Trainium2 kernel (8 NeuronCores, SPMD).

Problem: b=4, n1=n2=8192, dim=256, 8 heads x 64 dim_head.
  q = x @ Wq.T                     (rotary 2D on q)
  k, v = split(z @ Wkv.T)          (LayerNorm per head-dim on k, v; rotary on k)
  dots = k^T v / n2 per (b, h);  out = (q @ dots) @ Wo.T + bo

Sharding: flatten (b, n) -> 32768 rows; core c owns rows [4096c, 4096(c+1))
(one half of batch c//2).  The z-side partial k^T v is summed with a
pairwise AllReduce {0,1},{2,3},{4,5},{6,7}.

Design (v2):
  - Host pre-transposes x/z and converts inputs+weights+tables to fp16,
    so the kernel never runs PE transposes and all elementwise work runs
    in the DVE 2-byte fast path.
  - x side computes qT = Wq @ xT directly in transposed layout; the
    rotate-half term is a second projection with row-permuted Wq, so
    rotary is pure elementwise with partition-replicated tables.
  - z side is row-major (lhsT = zT chunks).  Wkv is host-centered (+
    gamma folded), so LayerNorm is sumsq -> rstd; rstd_k*rstd_v folds
    into v (trivial-affine case).  k rotary uses sign-baked sin tables
    and a stride trick for rotate-half.
  - kv / q PSUM results are copied once to SBUF fp16 by the scalar
    engine; every downstream elementwise op is then 2-byte packed SBUF
    (DVE 2x mode).  The LN reduce is a 2-level pairwise fold + short
    reduce.
  - dots accumulates transposed (lhsT = v2) into PSUM across all 32 row
    tiles; x-side work lags the z side by 16 iterations so the pairwise
    AllReduce overlaps the x tail (which avoids the gpsimd queue, where
    the collective is triggered).  dots is folded into Wo
    (W2 = blkdiag(dotsT) @ WoT * 1/n2), so the output stage is
    y = rotqT.T @ W2 + bo.
"""

import sys

sys.path.insert(0, "/opt/trn_rl_repo")

from contextlib import ExitStack

import numpy as np

import concourse.bass as bass
import concourse.tile as tile
from concourse import bacc, mybir
from concourse.bass import ts
from concourse.bass_utils import run_bass_kernel_spmd

F32 = mybir.dt.float32
F16 = mybir.dt.float16
ALU = mybir.AluOpType
AX = mybir.AxisListType
AF = mybir.ActivationFunctionType

B, N1, DIM = 4, 8192, 256
H, DH = 8, 64
INNER = H * DH  # 512
NCORES = 8
ROWS = (B * N1) // NCORES  # 4096 rows per core
NT = ROWS // 128  # 32 tiles of 128 rows
NG = ROWS // 512  # 8 x-side groups of 512 rows
LAG = 16  # x-side iteration lag behind z side
EPS = 1e-5

_nc_cache = {}


def _view(ap, offset, dims):
    """AP view with explicit free dims; keeps the partition dim."""
    return bass.AP(
        tensor=ap.tensor,
        offset=ap.offset + offset,
        ap=[list(ap.ap[0])] + [list(d) for d in dims],
    )


def build_nc(triv: bool):
    if triv in _nc_cache:
        return _nc_cache[triv]
    nc = bacc.Bacc(trn_type="TRN2", num_devices=NCORES, debug=False)

    zt_d = nc.dram_tensor("zt", [DIM, ROWS], F16, kind="ExternalInput").ap()
    xt_d = nc.dram_tensor("xt", [DIM, ROWS], F16, kind="ExternalInput").ap()
    wq_d = nc.dram_tensor("wq", [DIM, INNER], F16, kind="ExternalInput").ap()
    wqrh_d = nc.dram_tensor("wqrh", [DIM, INNER], F16, kind="ExternalInput").ap()
    wkv_d = nc.dram_tensor("wkv", [DIM, 2 * INNER], F16, kind="ExternalInput").ap()
    wo_d = nc.dram_tensor("wo", [INNER, DIM], F16, kind="ExternalInput").ap()
    bo_d = nc.dram_tensor("bo", [1, DIM], F32, kind="ExternalInput").ap()
    # k tables, row-major: [rows, 128] = cos(64, gamma_k folded) ||
    # sign-baked sin(64, gamma_k at swapped index folded)
    ktab_d = nc.dram_tensor("ktab", [ROWS, 128], F16, kind="ExternalInput").ap()
    # q tables, partition-replicated: [128, 2*rows] = cos || sign-sin
    qtab_d = nc.dram_tensor("qtab", [128, 2 * ROWS], F16, kind="ExternalInput").ap()
    if not triv:
        rbk_d = nc.dram_tensor("rbk", [ROWS, 64], F16, kind="ExternalInput").ap()
        bv_d = nc.dram_tensor("bv", [1, INNER], F32, kind="ExternalInput").ap()
    y_d = nc.dram_tensor("y", [ROWS, DIM], F32, kind="ExternalOutput").ap()
    cc_in = nc.dram_tensor("cc_in", [128, 512], F16)
    cc_out = nc.dram_tensor("cc_out", [128, 512], F16)

    with tile.TileContext(nc) as tc, ExitStack() as ctx:
        consts = ctx.enter_context(tc.tile_pool(name="consts", bufs=1))

        eps_t = consts.tile([128, 1], F32)
        nc.vector.memset(eps_t, EPS)

        wkv_t = []
        for kc in range(2):
            t = consts.tile([128, 2 * INNER], F16, tag=f"wkv{kc}")
            nc.sync.dma_start(t, wkv_d[ts(kc, 128), :])
            wkv_t.append(t)
        wq_t = []
        wqrh_t = []
        for kc in range(2):
            t = consts.tile([128, INNER], F16, tag=f"wq{kc}")
            nc.sync.dma_start(t, wq_d[ts(kc, 128), :])
            wq_t.append(t)
            t = consts.tile([128, INNER], F16, tag=f"wqrh{kc}")
            nc.sync.dma_start(t, wqrh_d[ts(kc, 128), :])
            wqrh_t.append(t)
        wo_t = []
        for p in range(4):
            t = consts.tile([128, DIM], F16, tag=f"wo{p}")
            nc.sync.dma_start(t, wo_d[ts(p, 128), :])
            wo_t.append(t)
        bo_bc = consts.tile([128, DIM], F32)
        nc.sync.dma_start(bo_bc, _view(bo_d, 0, [[1, DIM]]).partition_broadcast(128))
        if not triv:
            bv_bc = consts.tile([128, INNER], F32)
            nc.sync.dma_start(
                bv_bc, _view(bv_d, 0, [[1, INNER]]).partition_broadcast(128)
            )

        rotq_sb = []
        for c in range(4):
            rotq_c = consts.tile([128, ROWS], F16, tag=f"rotq{c}", name=f"rotq{c}")
            rotq_sb.append(rotq_c)

        dots_sb = consts.tile([128, 512], F16)
        dots_rd = consts.tile([128, 512], F16)
        blk = consts.tile([128, 512], F16)
        w2_sb = consts.tile([128, 1024], F16)

        def x_iter(j, zin, elem, stats, q_pp, use_gpsimd):
            """One x-side iteration: group g = j//4, inner chunk c = j%4."""
            g, c = j // 4, j % 4
            if c == 0:
                xc = []
                for kc in range(2):
                    t = zin.tile([128, 512], F16, tag=f"xc{kc}", name=f"xc{kc}")
                    nc.sync.dma_start(t, xt_d[ts(kc, 128), ts(g, 512)])
                    xc.append(t)
                qcos = zin.tile([128, 512], F16, tag="qcos")
                nc.sync.dma_start(qcos, qtab_d[:, ts(g, 512)])
                qsin = zin.tile([128, 512], F16, tag="qsin")
                nc.sync.dma_start(
                    qsin, qtab_d[:, ROWS + g * 512 : ROWS + (g + 1) * 512]
                )
                x_iter.cur = (xc, qcos, qsin)
            xc, qcos, qsin = x_iter.cur

            q_ps = q_pp.tile([128, 1024], F32, tag="q_ps")
            for kc in range(2):
                nc.tensor.matmul(
                    q_ps[:, 0:512],
                    wq_t[kc][:, ts(c, 128)],
                    xc[kc],
                    start=(kc == 0),
                    stop=(kc == 1),
                )
            for kc in range(2):
                nc.tensor.matmul(
                    q_ps[:, 512:1024],
                    wqrh_t[kc][:, ts(c, 128)],
                    xc[kc],
                    start=(kc == 0),
                    stop=(kc == 1),
                )
            q16 = elem.tile([128, 1024], F16, tag="q16")
            nc.scalar.copy(q16, q_ps)
            qcs = elem.tile([128, 512], F16, tag="qcs")
            qrs = elem.tile([128, 512], F16, tag="qrs")
            nc.vector.tensor_tensor(qcs, q16[:, 0:512], qcos, op=ALU.mult)
            if use_gpsimd:
                nc.gpsimd.tensor_tensor(qrs, q16[:, 512:1024], qsin, op=ALU.mult)
            else:
                nc.vector.tensor_tensor(qrs, q16[:, 512:1024], qsin, op=ALU.mult)
            nc.vector.tensor_tensor(
                rotq_sb[c][:, ts(g, 512)], qcs, qrs, op=ALU.add
            )

        # ------------- main loop: z side, x side lagging by LAG -------------
        with ExitStack() as pz:
            zin = pz.enter_context(tc.tile_pool(name="zin", bufs=2))
            tabs = pz.enter_context(tc.tile_pool(name="tabs", bufs=3))
            elem = pz.enter_context(tc.tile_pool(name="elem", bufs=2))
            stats = pz.enter_context(tc.tile_pool(name="stats", bufs=3))
            kv_pp = pz.enter_context(tc.tile_pool(name="kv_pp", bufs=1, space="PSUM"))
            q_pp = pz.enter_context(tc.tile_pool(name="q_pp", bufs=1, space="PSUM"))
            dots_pp = pz.enter_context(
                tc.tile_pool(name="dots_pp", bufs=1, space="PSUM")
            )

            dots_t = []
            for p in range(4):
                dots_t.append(
                    dots_pp.tile([128, 256], F32, tag=f"dots{p}", name=f"dots{p}")
                )

            prev = None  # (krot, v2) awaiting dots matmuls
            for i in range(NT):
                if i % 4 == 0:
                    zc = []
                    for kc in range(2):
                        t = zin.tile([128, 512], F16, tag=f"zc{kc}", name=f"zc{kc}")
                        nc.sync.dma_start(t, zt_d[ts(kc, 128), ts(i // 4, 512)])
                        zc.append(t)
                ktab_t = tabs.tile([128, 128], F16, tag="ktab")
                nc.sync.dma_start(ktab_t, ktab_d[ts(i, 128), :])
                if not triv:
                    rbk_t = tabs.tile([128, 64], F16, tag="rbk")
                    nc.sync.dma_start(rbk_t, rbk_d[ts(i, 128), :])

                kv_ps = kv_pp.tile([128, 1024], F32, tag="kv_ps")
                for kc in range(2):
                    nc.tensor.matmul(
                        kv_ps[:, 0:512],
                        zc[kc][:, ts(i % 4, 128)],
                        wkv_t[kc][:, 0:INNER],
                        start=(kc == 0),
                        stop=(kc == 1),
                    )
                for kc in range(2):
                    nc.tensor.matmul(
                        kv_ps[:, 512:1024],
                        zc[kc][:, ts(i % 4, 128)],
                        wkv_t[kc][:, INNER : 2 * INNER],
                        start=(kc == 0),
                        stop=(kc == 1),
                    )

                # lagged x-side iteration (keeps PE busy while DVE catches up)
                if i >= LAG:
                    x_iter(i - LAG, zin, elem, stats, q_pp, use_gpsimd=True)

                # dots for the previous tile (gives DVE a full iteration)
                if prev is not None:
                    pk, pv = prev
                    for p in range(4):
                        nc.tensor.matmul(
                            dots_t[p],
                            pv[:, ts(p, 128)],
                            pk[:, (p // 2) * 256 : (p // 2) * 256 + 256],
                            start=(i == 1),
                            stop=False,
                        )

                kv16 = elem.tile([128, 2 * INNER], F16, tag="kv16")
                nc.scalar.copy(kv16, kv_ps)

                sq = elem.tile([128, 2 * INNER], F16, tag="sq")
                nc.vector.tensor_tensor(sq, kv16, kv16, op=ALU.mult)
                f1 = elem.tile([128, INNER], F16, tag="f1")
                nc.vector.tensor_tensor(
                    _view(f1, 0, [[32, 16], [1, 32]]),
                    _view(sq, 0, [[64, 16], [1, 32]]),
                    _view(sq, 32, [[64, 16], [1, 32]]),
                    op=ALU.add,
                )
                f2 = elem.tile([128, 256], F16, tag="f2")
                nc.vector.tensor_tensor(
                    _view(f2, 0, [[16, 16], [1, 16]]),
                    _view(f1, 0, [[32, 16], [1, 16]]),
                    _view(f1, 16, [[32, 16], [1, 16]]),
                    op=ALU.add,
                )
                sums = stats.tile([128, 16], F32, tag="sums")
                nc.vector.reduce_sum(
                    sums, _view(f2, 0, [[16, 16], [1, 16]]), axis=AX.X
                )
                std = stats.tile([128, 16], F32, tag="std")
                nc.scalar.activation(
                    std, sums, AF.Sqrt, scale=1.0 / DH, bias=eps_t[:, 0:1]
                )
                rstd = stats.tile([128, 16], F32, tag="rstd")
                nc.vector.reciprocal(rstd, std)

                # rotary on k (fp16 fast path): kcs = k*cos, krs = swap(k)*ssin
                kcs = elem.tile([128, INNER], F16, tag="kcs")
                krs = elem.tile([128, INNER], F16, tag="krs")
                krot = elem.tile([128, INNER], F16, tag="krot")
                v2 = elem.tile([128, INNER], F16, tag="v2")
                nc.vector.tensor_tensor(
                    _view(kcs, 0, [[64, 8], [32, 2], [16, 2], [1, 16]]),
                    _view(kv16, 0, [[64, 8], [32, 2], [16, 2], [1, 16]]),
                    _view(ktab_t, 0, [[0, 8], [32, 2], [16, 2], [1, 16]]),
                    op=ALU.mult,
                )
                nc.vector.tensor_tensor(
                    _view(krs, 0, [[64, 8], [32, 2], [16, 2], [1, 16]]),
                    _view(kv16, 16, [[64, 8], [32, 2], [-16, 2], [1, 16]]),
                    _view(ktab_t, 64, [[0, 8], [32, 2], [16, 2], [1, 16]]),
                    op=ALU.mult,
                )
                if triv:
                    rkv = stats.tile([128, 8], F32, tag="rkv")
                    nc.vector.tensor_tensor(
                        rkv, rstd[:, 0:8], rstd[:, 8:16], op=ALU.mult
                    )
                    nc.gpsimd.tensor_tensor(
                        _view(v2, 0, [[64, 8], [1, 64]]),
                        _view(kv16, 512, [[64, 8], [1, 64]]),
                        _view(rkv, 0, [[1, 8], [0, 64]]),
                        op=ALU.mult,
                    )
                    nc.vector.tensor_tensor(krot, kcs, krs, op=ALU.add)
                else:
                    # krot = rstd_k * (kcs + krs) + rot(beta_k)
                    t1 = elem.tile([128, INNER], F16, tag="t1")
                    nc.vector.tensor_tensor(t1, kcs, krs, op=ALU.add)
                    t2 = elem.tile([128, INNER], F16, tag="t2")
                    nc.vector.tensor_tensor(
                        _view(t2, 0, [[64, 8], [1, 64]]),
                        _view(t1, 0, [[64, 8], [1, 64]]),
                        _view(rstd, 0, [[1, 8], [0, 64]]),
                        op=ALU.mult,
                    )
                    nc.gpsimd.tensor_tensor(
                        _view(krot, 0, [[64, 8], [1, 64]]),
                        _view(t2, 0, [[64, 8], [1, 64]]),
                        _view(rbk_t, 0, [[0, 8], [1, 64]]),
                        op=ALU.add,
                    )
                    # v2 = rstd_v * v + beta_v (gamma_v folded into Wkv)
                    t3 = elem.tile([128, INNER], F16, tag="t3")
                    nc.gpsimd.tensor_tensor(
                        _view(t3, 0, [[64, 8], [1, 64]]),
                        _view(kv16, 512, [[64, 8], [1, 64]]),
                        _view(rstd, 8, [[1, 8], [0, 64]]),
                        op=ALU.mult,
                    )
                    nc.vector.tensor_tensor(v2, t3, bv_bc, op=ALU.add)
                prev = (krot, v2)

            # final dots tile
            pk, pv = prev
            for p in range(4):
                nc.tensor.matmul(
                    dots_t[p],
                    pv[:, ts(p, 128)],
                    pk[:, (p // 2) * 256 : (p // 2) * 256 + 256],
                    start=False,
                    stop=True,
                )
            for p in range(4):
                nc.vector.tensor_copy(
                    dots_sb[:, ts(p, 128)],
                    dots_t[p][:, (p % 2) * 128 : (p % 2) * 128 + 128],
                )
            nc.sync.dma_start(cc_in.ap(), dots_sb)
            nc.gpsimd.collective_compute(
                "AllReduce",
                ALU.add,
                replica_groups=[[0, 1], [2, 3], [4, 5], [6, 7]],
                ins=[cc_in.ap()],
                outs=[cc_out.ap()],
            )

            # x tail: iterations LAG..NT-1 overlap the AllReduce (no gpsimd)
            for j in range(NT - LAG, NT):
                x_iter(j, zin, elem, stats, q_pp, use_gpsimd=False)

        # ---- dots -> W2 = blkdiag(dotsT) @ WoT / n2 ----
        with ExitStack() as pw:
            w2_pp = pw.enter_context(tc.tile_pool(name="w2_pp", bufs=1, space="PSUM"))
            nc.sync.dma_start(dots_rd, cc_out.ap())
            nc.vector.memset(blk, 0.0)
            for p in range(4):
                nc.vector.tensor_copy(
                    blk[0:64, p * 128 : p * 128 + 64],
                    dots_rd[0:64, p * 128 : p * 128 + 64],
                )
                nc.vector.tensor_copy(
                    blk[64:128, p * 128 + 64 : p * 128 + 128],
                    dots_rd[64:128, p * 128 + 64 : p * 128 + 128],
                )
            for p in range(4):
                w2_ps = w2_pp.tile([128, 256], F32, tag=f"w2_{p}", name=f"w2_{p}")
                nc.tensor.matmul(w2_ps, blk[:, ts(p, 128)], wo_t[p])
                nc.scalar.mul(w2_sb[:, ts(p, 256)], w2_ps, 1.0 / N1)

        # ---------------- phase Y: y = rotqT.T @ W2 + bo ----------------
        with ExitStack() as py:
            outp = py.enter_context(tc.tile_pool(name="outp", bufs=3))
            y_pp = py.enter_context(tc.tile_pool(name="y_pp", bufs=2, space="PSUM"))

            for t in range(NT):
                y_ps = y_pp.tile([128, DIM], F32, tag="y_ps")
                for c in range(4):
                    nc.tensor.matmul(
                        y_ps,
                        rotq_sb[c][:, ts(t, 128)],
                        w2_sb[:, ts(c, 256)],
                        start=(c == 0),
                        stop=(c == 3),
                    )
                y_sb = outp.tile([128, DIM], F32, tag="y_sb")
                nc.vector.tensor_tensor(y_sb, y_ps, bo_bc, op=ALU.add)
                nc.sync.dma_start(y_d[ts(t, 128), :], y_sb)

    nc.compile()
    _nc_cache[triv] = nc
    return nc


def _freqs(pos):
    """pos [rows, 2] -> f [rows, 32] = 16 x-freqs || 16 y-freqs."""
    rdim = DH // 2  # 32
    inv_freq = (
        1.0 / (10000.0 ** (np.arange(0, rdim, 2, dtype=np.float64) / rdim))
    ).astype(np.float64)  # [16]
    t = pos.astype(np.float64) * 64.0  # SCALE / MIN_FREQ
    return np.concatenate([t[:, 0:1] * inv_freq, t[:, 1:2] * inv_freq], axis=1)


# inner-dim helpers: d (0..63 within head) = a*32 + qd*16 + e
_d = np.arange(64)
_a = _d // 32
_qd = (_d % 32) // 16
_e = _d % 16
_freq_idx = _a * 16 + _e  # [64] -> col in [0,32)
_sign = np.where(_qd == 0, -1.0, 1.0)  # quad0: -sin, quad1: +sin
_swap = _d + 16 - 32 * _qd  # rotate-half partner within head


def _ktables(pos, k_gamma):
    """k-side row-major tables [rows, 128]: cos*gamma || sign*sin*gamma_swap."""
    f = _freqs(pos)  # [rows, 32]
    g = np.asarray(k_gamma, dtype=np.float64)
    cos = np.cos(f)[:, _freq_idx] * g[None, :]
    sin = np.sin(f)[:, _freq_idx] * _sign[None, :] * g[_swap][None, :]
    return np.ascontiguousarray(
        np.concatenate([cos, sin], axis=1).astype(np.float16)
    )


def _qtables(pos):
    """q-side partition-replicated tables [128, 2*rows]: cos || sign*sin."""
    f = _freqs(pos)  # [rows, 32]
    cosT = np.cos(f).T  # [32, rows]
    sinT = np.sin(f).T
    d128 = np.arange(128) % 64
    cos_rep = cosT[_freq_idx[d128], :]  # [128, rows]
    sin_rep = sinT[_freq_idx[d128], :] * _sign[d128][:, None]
    return np.ascontiguousarray(
        np.concatenate([cos_rep, sin_rep], axis=1).astype(np.float16)
    )


def _rbk_table(pos, k_beta):
    """rot(beta_k) [rows, 64] for the non-trivial-affine path."""
    f = _freqs(pos)
    b = np.asarray(k_beta, dtype=np.float64)
    return np.ascontiguousarray(
        (
            np.cos(f)[:, _freq_idx] * b[None, :]
            + np.sin(f)[:, _freq_idx] * _sign[None, :] * b[_swap][None, :]
        ).astype(np.float16)
    )


def _prepare(
    x, z, x_pos, z_pos, Wq, Wkv, k_gamma, k_beta, v_gamma, v_beta, Wo, bo
):
    """Host prep: returns (nc, in_maps) ready for run_bass_kernel_spmd."""
    xf = np.asarray(x, dtype=np.float32).reshape(B * N1, DIM)
    zf = np.asarray(z, dtype=np.float32).reshape(B * N1, DIM)
    xpf = np.asarray(x_pos).reshape(B * N1, 2)
    zpf = np.asarray(z_pos).reshape(B * N1, 2)

    triv = bool(
        np.all(np.asarray(k_gamma) == 1.0)
        and np.all(np.asarray(k_beta) == 0.0)
        and np.all(np.asarray(v_gamma) == 1.0)
        and np.all(np.asarray(v_beta) == 0.0)
    )

    wqT = np.asarray(Wq, dtype=np.float64).T  # [256, 512]
    j = np.arange(INNER)
    swap_full = (j // 64) * 64 + _swap[j % 64]
    wqrhT = np.ascontiguousarray(wqT[:, swap_full].astype(np.float16))
    wqT = np.ascontiguousarray(wqT.astype(np.float16))

    # Wkv: center per 64-col head block (exact mean removal), fold gamma_v
    # into the v half.
    wkvT = np.asarray(Wkv, dtype=np.float64).T  # [256, 1024]
    wkv_c = wkvT.reshape(DIM, 16, DH)
    wkv_c = wkv_c - wkv_c.mean(axis=2, keepdims=True)
    if not triv:
        gv = np.asarray(v_gamma, dtype=np.float64)
        wkv_c[:, 8:16, :] = wkv_c[:, 8:16, :] * gv[None, None, :]
    wkvT = np.ascontiguousarray(wkv_c.reshape(DIM, 2 * INNER).astype(np.float16))

    woT = np.ascontiguousarray(np.asarray(Wo).T.astype(np.float16))  # [512, 256]
    bo_r = np.ascontiguousarray(np.asarray(bo).reshape(1, DIM)).astype(np.float32)

    kg = np.asarray(k_gamma, dtype=np.float64) if not triv else np.ones(DH)

    nc = build_nc(triv)

    in_maps = []
    for c in range(NCORES):
        lo, hi = c * ROWS, (c + 1) * ROWS
        m = {
            "zt": np.ascontiguousarray(zf[lo:hi].T.astype(np.float16)),
            "xt": np.ascontiguousarray(xf[lo:hi].T.astype(np.float16)),
            "wq": wqT,
            "wqrh": wqrhT,
            "wkv": wkvT,
            "wo": woT,
            "bo": bo_r,
            "ktab": _ktables(zpf[lo:hi], kg),
            "qtab": _qtables(xpf[lo:hi]),
        }
        if not triv:
            m["rbk"] = _rbk_table(zpf[lo:hi], k_beta)
            m["bv"] = np.ascontiguousarray(
                np.tile(np.asarray(v_beta), H)[None, :]
            ).astype(np.float32)
        in_maps.append(m)
    return nc, in_maps


def kernel(**inputs):
    nc, in_maps = _prepare(**inputs)
    res = run_bass_kernel_spmd(nc, in_maps, list(range(NCORES)))
    y = np.concatenate([res.results[c]["y"] for c in range(NCORES)], axis=0)
    return y.reshape(B, N1, DIM).astype(np.float32)
